# revision 1
# baseline (speedup 1.0000x reference)
"""Trainium2 Bass kernel for nn_BinaryTokenClassificationModel (segment_reduce).

Math: the reference pools token embeddings into word embeddings (mean over
contiguous runs of equal word ids), then computes
    logits[b,s,t] = src_pooled[b,s] @ w_src + tgt_pooled[b,t] @ w_tgt + b.
Because the classifier is linear, pooling and projection commute:
    src_proj[w] = sum_t A[w,t] * (tok_h[t] @ w_src)     (A = 1/count-weighted
    tgt_proj[w] = sum_t A[w,t] * (tok_h[t] @ w_tgt)      segment membership)
and the output is the outer sum src_proj[s] + tgt_proj[t] + b. Each core:
  1. streams its batch row of tok_h [512, 768] through a fused DVE
     multiply-reduce against the broadcast weight row -> u[t] (per-token scalar)
  2. builds the membership matrix on-device (GpSimd iota + compare against the
     per-token segment index) -- no membership DMA traffic
  3. accumulates  atw_c.T @ broadcast(u_c)  (src chunks) and
     broadcast(u_c).T @ atw_c  (tgt chunks) straight into the [S, T] output
     PSUM tile, which realizes segment-reduce + outer-sum in one matmul/chunk.
Data-parallel over batch: core i handles batch row i. No collectives.
"""

import functools

import numpy as np

import concourse.bacc as bacc
import concourse.mybir as mybir
from concourse.bass_utils import run_bass_kernel_spmd
from concourse.tile import TileContext
from concourse.tile_rust import add_dep_helper

# Problem geometry (hardcoded per spec)
B = 8
L_SRC = 256
L_TGT = 256
L = L_SRC + L_TGT  # 512
H = 768
P = 128            # SBUF partitions / tokens per chunk
NCHUNK = L // P    # 4
N_SRC_CHUNKS = L_SRC // P  # 2
N_CORES = 8
F32 = mybir.dt.float32


# ---------------------------------------------------------------------------
# Host-side segment bookkeeping (exact mirror of reference._pool_words)
# ---------------------------------------------------------------------------

def _segments(combined_wid, attention_mask, n_words):
    """Per-token dense run ids exactly as the reference computes them."""
    valid = (attention_mask > 0) & (combined_wid >= 0)  # [B, L]
    prev_wid = np.concatenate(
        [np.full((combined_wid.shape[0], 1), -2, dtype=combined_wid.dtype),
         combined_wid[:, :-1]], axis=1)
    prev_valid = np.concatenate(
        [np.zeros((valid.shape[0], 1), dtype=bool), valid[:, :-1]], axis=1)
    new_run = valid & ((combined_wid != prev_wid) | (~prev_valid))
    run_id = np.cumsum(new_run.astype(np.int64), axis=1) - 1  # [B, L]
    seg = np.where(valid, run_id, n_words)  # n_words = dummy slot
    return seg, valid


def _seg_weights(seg, valid, n_words):
    """1/max(count,1) weight for each token's segment (0 for invalid)."""
    Bv, Lv = seg.shape
    wgt = np.zeros((Bv, Lv), dtype=np.float32)
    for b in range(Bv):
        counts = np.bincount(seg[b][valid[b]], minlength=Lv + 1).astype(np.float32)
        inv = 1.0 / np.maximum(counts, 1.0)
        wgt[b] = np.where(valid[b] & (seg[b] < n_words), inv[np.minimum(seg[b], Lv)], 0.0)
    return wgt


def _host_forward(tok_h, attention_mask, source_word_ids, target_word_ids, W, b, S, T):
    """Pure numpy forward implementing the same algebra the device runs.

    Used for validation only (test harness); not called by kernel().
    """
    combined = np.concatenate([source_word_ids, target_word_ids], axis=1).astype(np.int64)
    seg, valid = _segments(combined, np.asarray(attention_mask), S + T)
    wgt = _seg_weights(seg, valid, S + T)
    w_src = W[:H, 0].astype(np.float32)
    w_tgt = W[H:2 * H, 0].astype(np.float32)
    out = np.empty((tok_h.shape[0], S, T), dtype=np.float32)
    for bi in range(tok_h.shape[0]):
        u_src = tok_h[bi].astype(np.float32) @ w_src  # [L]
        u_tgt = tok_h[bi].astype(np.float32) @ w_tgt  # [L]
        proj = np.zeros(S + T, dtype=np.float32)
        for t in range(L):
            s = seg[bi, t]
            if s < S:
                proj[s] += wgt[bi, t] * u_src[t]
            elif s < S + T:
                proj[s] += wgt[bi, t] * u_tgt[t]
        out[bi] = proj[:S, None] + proj[None, S:S + T] + float(np.asarray(b).reshape(-1)[0])
    return out


# ---------------------------------------------------------------------------
# Device kernel, fast path (block_ok): src tokens -> word rows [0,S),
# tgt tokens -> word rows [S,S+T)
# ---------------------------------------------------------------------------

def _declare_block_params(nc, S, T):
    MW = 2 * NCHUNK  # meta columns
    return dict(
        tok0=nc.declare_dram_parameter("tok0", [P, H + MW], F32, isOutput=False),
        tok1=nc.declare_dram_parameter("tok1", [P, H], F32, isOutput=False),
        tok2=nc.declare_dram_parameter("tok2", [P, H], F32, isOutput=False),
        tok3=nc.declare_dram_parameter("tok3", [P, H], F32, isOutput=False),
        # wcat = [w_src (H) | w_tgt (H) | bias (1)]
        wcat=nc.declare_dram_parameter("wcat", [1, 2 * H + 1], F32, isOutput=False),
        # iotac[p, w] = w  (constant; GpSimd iota is a slow SW op on HW)
        iotac=nc.declare_dram_parameter("iotac", [P, P], F32, isOutput=False),
        out=nc.declare_dram_parameter("out", [S, T], F32, isOutput=True),
    )


def _emit_block_body(nc, tc, prm, S, T, mm_mode="mat", prod_space="SBUF"):
    """Fast path. DMA layout: chunk 0 and chunk 3 token loads are split in
    half-rows -- chunk 0 so the (tiny, pipeline-gating) wcat transfer can slip
    into the DMA stream between the halves, chunk 3 so the tail reduce after
    the last byte lands is half-length. meta rides as extra columns packed
    into the first token piece (no DMA of its own)."""
    MW = 2 * NCHUNK
    tok0, tok1, tok2, tok3, wcat, iotac, out = (
        prm["tok0"], prm["tok1"], prm["tok2"], prm["tok3"],
        prm["wcat"], prm["iotac"], prm["out"])
    if True:
        with (
            tc.tile_pool(name="const", bufs=1) as cpool,
            tc.tile_pool(name="toks", bufs=6) as tpool,
            tc.tile_pool(name="prods", bufs=2) as ppool,
            tc.tile_pool(name="atws", bufs=2) as apool,
            tc.tile_pool(name="psum", bufs=1, space="PSUM") as pspool,
        ):
            # wcat rides the SWDGE (Pool) path so it never contends with the
            # token loads for HWDGE issue slots; it gates the weight
            # broadcasts which gate the whole DVE pipeline.
            with tc.high_priority():
                wcat_sb = cpool.tile([1, 2 * H + 1], F32)
                nc.scalar.dma_start(out=wcat_sb[:], in_=wcat[:])

            # token chunk loads own the SP HWDGE queue (~75% of all bytes)
            t0 = tpool.tile([P, H + MW], F32)
            nc.sync.dma_start(out=t0[:], in_=tok0[:])
            t1 = tpool.tile([P, H], F32)
            nc.sync.dma_start(out=t1[:], in_=tok1[:])
            t2 = tpool.tile([P, H], F32)
            nc.sync.dma_start(out=t2[:], in_=tok2[:])
            t3 = tpool.tile([P, H], F32)
            nc.sync.dma_start(out=t3[:], in_=tok3[:])
            meta_sb = t0[:, H:H + MW]

            # weight rows broadcast across partitions (GpSimd cross-partition
            # op; overlaps the token DMA stream)
            wb_src = cpool.tile([P, H], F32)
            wb_tgt = cpool.tile([P, H], F32)
            with tc.high_priority():
                nc.gpsimd.partition_broadcast(wb_src[:], wcat_sb[0:1, 0:H])
                nc.gpsimd.partition_broadcast(wb_tgt[:], wcat_sb[0:1, H:2 * H])

            # iota constant rides the idle ACT DGE queue
            iota_f = cpool.tile([P, P], F32)
            nc.scalar.dma_start(out=iota_f[:], in_=iotac[:])

            # bias column [S, 1]: broadcast b down the partitions (Pool,
            # off the critical path); added during the final copy-out
            bias_col = cpool.tile([P, 1], F32)
            nc.gpsimd.partition_broadcast(bias_col[:], wcat_sb[0:1, 2 * H:2 * H + 1])

            if mm_mode == "mat":
                ones_pt = cpool.tile([P, P], F32)
                nc.vector.memset(ones_pt[:], 1.0)

            psum_out = pspool.tile([S, T], F32)

            # per-chunk token pieces: (tile, column range) pairs
            chunk_pieces = [
                [(t0, 0, H)],
                [(t1, 0, H)],
                [(t2, 0, H)],
                [(t3, 0, H)],
            ]

            # membership tiles: atw_c[t, w] = (seg[t] == w) * wgt[t].
            # Built on DVE (GpSimd tensor_scalar is ~2.4us/op on HW), before
            # the reduce chain so they are off the critical tail.
            atw_tiles = []
            for c in range(NCHUNK):
                width = S if c < N_SRC_CHUNKS else T
                atw_c = apool.tile([P, P], F32, name=f"atw_{c}")
                nc.vector.tensor_scalar(
                    out=atw_c[:, :width], in0=iota_f[:, :width],
                    scalar1=meta_sb[:, 2 * c:2 * c + 1],
                    scalar2=meta_sb[:, 2 * c + 1:2 * c + 2],
                    op0=mybir.AluOpType.is_equal, op1=mybir.AluOpType.mult)
                atw_tiles.append(atw_c)

            u_sb = cpool.tile([P, 2 * NCHUNK], F32)
            scratch_col = NCHUNK
            for c in range(NCHUNK):
                is_src = c < N_SRC_CHUNKS
                width = S if is_src else T
                wb = wb_src if is_src else wb_tgt

                # u_c[t] = tok_c[t, :] . w  -- fused multiply+reduce on DVE
                # (AFFINE_MUL_REDUCE custom op; seed is 0 so multi-piece
                # chunks sum their partials with one [P,1] add)
                pieces = chunk_pieces[c]
                accs = []
                for pi, (tile_, j0, j1) in enumerate(pieces):
                    if len(pieces) == 1:
                        acc = u_sb[:, c:c + 1]
                    else:
                        acc = u_sb[:, scratch_col:scratch_col + 1]
                        scratch_col += 1
                    prod = ppool.tile([P, j1 - j0], F32, name=f"prod_{c}_{pi}",
                                      space=prod_space)
                    nc.vector.affine_mul_reduce(
                        out=prod[:], accum_out=acc, in0=tile_[:, 0:j1 - j0],
                        in1=wb[:, j0:j1], scale=1.0, bias=0.0)
                    accs.append(acc)
                if len(accs) > 1:
                    nc.vector.tensor_tensor(
                        out=u_sb[:, c:c + 1], in0=accs[0], in1=accs[1],
                        op=mybir.AluOpType.add)

                atw_c = atw_tiles[c]
                u_b = u_sb[:, c:c + 1]
                if mm_mode == "mat":
                    ub_mat = ppool.tile([P, P], F32, name=f"ubm_{c}", tag="ubm")
                    nc.vector.tensor_scalar_mul(ub_mat[:], ones_pt[:], u_b)
                    rhs_b, lhs_b = ub_mat[:, :T], ub_mat[:, :S]
                else:
                    rhs_b, lhs_b = u_b.broadcast_to([P, T]), u_b.broadcast_to([P, S])
                first = c == 0
                last = c == NCHUNK - 1
                if is_src:
                    # psum[s, t] += sum_t' atw[t', s] * u[t']  (same for all t)
                    nc.tensor.matmul(
                        psum_out[:], atw_c[:, :S], rhs_b,
                        start=first, stop=last)
                else:
                    nc.tensor.matmul(
                        psum_out[:], lhs_b, atw_c[:, :T],
                        start=first, stop=last)

            out_sb = cpool.tile([S, T], F32)
            nc.vector.tensor_scalar_add(out_sb[:], psum_out[:], bias_col[0:S, :])
            nc.sync.dma_start(out=out[:], in_=out_sb[:])


# ---------------------------------------------------------------------------
# Device kernel, general fallback: tokens may map into either word block
# ---------------------------------------------------------------------------

def _build_general(nc, S, T):
    NW = S + T
    tok = nc.declare_dram_parameter("tok", [L, H], F32, isOutput=False)
    atw = nc.declare_dram_parameter("atw", [NCHUNK, P, NW], F32, isOutput=False)
    wcat = nc.declare_dram_parameter("wcat", [1, 2 * H + 1], F32, isOutput=False)
    out = nc.declare_dram_parameter("out", [S, T], F32, isOutput=True)

    with TileContext(nc) as tc:
        with (
            tc.tile_pool(name="const", bufs=1) as cpool,
            tc.tile_pool(name="toks", bufs=3) as tpool,
            tc.tile_pool(name="prods", bufs=2) as ppool,
            tc.tile_pool(name="atws", bufs=2) as apool,
            tc.tile_pool(name="psum", bufs=1, space="PSUM") as pspool,
        ):
            wcat_sb = cpool.tile([1, 2 * H + 1], F32)
            nc.gpsimd.dma_start(out=wcat_sb[:], in_=wcat[:])
            ones = cpool.tile([1, P], F32)
            nc.vector.memset(ones[:], 1.0)
            bias_row = cpool.tile([1, T], F32)
            nc.vector.tensor_scalar_mul(
                bias_row[:], ones[:, :T], wcat_sb[0:1, 2 * H:2 * H + 1])

            wb_src = pspool.tile([P, H], F32)
            wb_tgt = pspool.tile([P, H], F32)
            for wb, w0 in ((wb_src, 0), (wb_tgt, H)):
                for j0, j1 in ((0, 512), (512, H)):
                    nc.tensor.matmul(
                        wb[:, j0:j1], ones[:, :P], wcat_sb[0:1, w0 + j0:w0 + j1],
                        start=True, stop=True)

            psum_out = pspool.tile([S, T], F32)
            nc.tensor.matmul(psum_out[:], ones[:, :S], bias_row[:],
                             start=True, stop=False)

            u_src_sb = cpool.tile([P, NCHUNK], F32)
            u_tgt_sb = cpool.tile([P, NCHUNK], F32)
            for c in range(NCHUNK):
                tok_c = tpool.tile([P, H], F32, name=f"tok_{c}")
                nc.sync.dma_start(out=tok_c[:], in_=tok[c * P:(c + 1) * P, :])
                for kind, wb, usb in (("s", wb_src, u_src_sb), ("t", wb_tgt, u_tgt_sb)):
                    prod = ppool.tile([P, H], F32, name=f"prod_{kind}_{c}")
                    nc.vector.affine_mul_reduce(
                        out=prod[:], accum_out=usb[:, c:c + 1], in0=tok_c[:],
                        in1=wb[:], scale=1.0, bias=0.0)

                atw_c = apool.tile([P, NW], F32, name=f"atw_{c}")
                nc.sync.dma_start(out=atw_c[:], in_=atw[c])
                last = c == NCHUNK - 1
                nc.tensor.matmul(
                    psum_out[:], atw_c[:, :S], u_src_sb[:, c:c + 1].broadcast_to([P, T]),
                    start=False, stop=False)
                nc.tensor.matmul(
                    psum_out[:], u_tgt_sb[:, c:c + 1].broadcast_to([P, S]), atw_c[:, S:],
                    start=False, stop=last)

            out_sb = cpool.tile([S, T], F32)
            nc.vector.tensor_scalar_add(out_sb[:], psum_out[:], bias_col[0:S, :])
            nc.sync.dma_start(out=out[:], in_=out_sb[:])


# variant knobs (fixed at import for the graded path; bench overrides)
MM_MODE = "mat"
PROD_SPACE = "SBUF"


@functools.lru_cache(maxsize=4)
def _build(S, T, block_ok, mm_mode=None, prod_space=None):
    mm_mode = MM_MODE if mm_mode is None else mm_mode
    prod_space = PROD_SPACE if prod_space is None else prod_space
    nc = bacc.Bacc("TRN2", debug=False, num_devices=N_CORES)
    if block_ok:
        prm = _declare_block_params(nc, S, T)
        with TileContext(nc) as tc:
            _emit_block_body(nc, tc, prm, S, T, mm_mode, prod_space)
    else:
        _build_general(nc, S, T)
    nc.compile()
    return nc


@functools.lru_cache(maxsize=16)
def _build_looped(S, T, iters, mm_mode=None, prod_space=None):
    """Timing-only variant: the same body repeated `iters` times inside one
    NEFF via a Tile For_i loop (per-iteration all-engine barrier back-edge)."""
    mm_mode = MM_MODE if mm_mode is None else mm_mode
    prod_space = PROD_SPACE if prod_space is None else prod_space
    nc = bacc.Bacc("TRN2", debug=False, num_devices=N_CORES)
    prm = _declare_block_params(nc, S, T)
    with TileContext(nc) as tc:
        with tc.For_i(0, iters, 1):
            _emit_block_body(nc, tc, prm, S, T, mm_mode, prod_space)
    nc.compile()
    return nc


# ---------------------------------------------------------------------------
# Host wrapper
# ---------------------------------------------------------------------------

def _prep(inputs):
    tok_h = np.ascontiguousarray(np.asarray(inputs["tok_h"], dtype=np.float32))
    mask = np.asarray(inputs["attention_mask"])
    swid = np.asarray(inputs["source_word_ids"])
    twid = np.asarray(inputs["target_word_ids"])
    W = np.asarray(inputs["W"], dtype=np.float32)
    b = np.asarray(inputs["b"], dtype=np.float32)
    S = int(np.asarray(inputs["S"]))
    T = int(np.asarray(inputs["T"]))

    Bv, Lv, Hv = tok_h.shape
    assert (Bv, Lv, Hv) == (B, L, H), f"unexpected tok_h shape {tok_h.shape}"
    assert swid.shape == (B, L_SRC) and twid.shape == (B, L_TGT)
    assert S <= P and T <= P

    NW = S + T
    combined = np.concatenate([swid, twid], axis=1).astype(np.int64)
    seg, valid = _segments(combined, mask, NW)
    wgt = _seg_weights(seg, valid, NW)

    src_tok_seg = seg[:, :L_SRC][valid[:, :L_SRC]]
    tgt_tok_seg = seg[:, L_SRC:][valid[:, L_SRC:]]
    block_ok = bool(
        (src_tok_seg < S).all()
        and (tgt_tok_seg >= S).all() and (tgt_tok_seg < NW).all()
    )

    wcat = np.zeros((1, 2 * H + 1), dtype=np.float32)
    wcat[0, :H] = W[:H, 0]
    wcat[0, H:2 * H] = W[H:2 * H, 0]
    wcat[0, 2 * H] = b.reshape(-1)[0]

    in_maps = []
    if block_ok:
        # meta[b, t_local, 2c] = in-block segment col (or -1), [.., 2c+1] = wgt
        meta = np.zeros((B, P, 2 * NCHUNK), dtype=np.float32)
        for bi in range(B):
            for c in range(NCHUNK):
                tsl = slice(c * P, (c + 1) * P)
                segc = seg[bi, tsl].astype(np.int64)
                col = segc if c < N_SRC_CHUNKS else segc - S
                ok = valid[bi, tsl] & (segc < NW)
                meta[bi, :, 2 * c] = np.where(ok, col, -1).astype(np.float32)
                meta[bi, :, 2 * c + 1] = wgt[bi, tsl]
        for i in range(N_CORES):
            bi = i % B
            tk = tok_h[bi]
            in_maps.append({
                # chunk 0 carries meta as extra columns
                "tok0": np.ascontiguousarray(
                    np.concatenate([tk[0:P, :], meta[bi]], axis=1)),
                "tok1": np.ascontiguousarray(tk[P:2 * P, :]),
                "tok2": np.ascontiguousarray(tk[2 * P:3 * P, :]),
                "tok3": np.ascontiguousarray(tk[3 * P:4 * P, :]),
                "wcat": wcat,
                "iotac": np.tile(np.arange(P, dtype=np.float32), (P, 1)),
            })
    else:
        atw = np.zeros((B, NCHUNK, P, NW), dtype=np.float32)
        for bi in range(B):
            for t in range(L):
                s = seg[bi, t]
                if s >= NW or not valid[bi, t]:
                    continue
                atw[bi, t // P, t % P, s] = wgt[bi, t]
        for i in range(N_CORES):
            bi = i % B
            in_maps.append({"tok": tok_h[bi], "atw": atw[bi], "wcat": wcat})
    return S, T, block_ok, in_maps


def kernel(**inputs):
    S, T, block_ok, in_maps = _prep(inputs)
    nc = _build(S, T, block_ok)
    res = run_bass_kernel_spmd(nc, in_maps, core_ids=list(range(N_CORES)))
    return np.stack([res.results[i]["out"] for i in range(B)], axis=0)


@functools.lru_cache(maxsize=4)
def _build_looped_empty(iters):
    """Calibration: same For_i loop with a minimal body, to measure the
    per-iteration loop overhead (back-edge barrier + sem reset)."""
    nc = bacc.Bacc("TRN2", debug=False, num_devices=N_CORES)
    x = nc.declare_dram_parameter("x", [P, 16], F32, isOutput=False)
    y = nc.declare_dram_parameter("y", [P, 16], F32, isOutput=True)
    with TileContext(nc) as tc:
        with tc.tile_pool(name="p", bufs=2) as pool:
            t = pool.tile([P, 16], F32)
            nc.sync.dma_start(out=t[:], in_=x[:])
            with tc.For_i(0, iters, 1):
                w = pool.tile([P, 16], F32)
                nc.vector.tensor_copy(w[:], t[:])
            nc.sync.dma_start(out=y[:], in_=t[:])
    nc.compile()
    return nc



# revision 2
# speedup vs baseline: 1.4163x; 1.4163x over previous
"""Trainium2 Bass kernel for nn_BinaryTokenClassificationModel (segment_reduce).

Math: the reference pools token embeddings into word embeddings (mean over
contiguous runs of equal word ids), then computes
    logits[b,s,t] = src_pooled[b,s] @ w_src + tgt_pooled[b,t] @ w_tgt + b.
Because the classifier is linear, pooling and projection commute:
    u[t]    = tok_h[t] @ w_blk(t)              (per-token scalar projection)
    proj[w] = sum_t atw[t, w] * u[t]           (atw = 1/count-weighted
                                                segment membership)
    logits[s, t] = proj_src[s] + proj_tgt[t] + b   (outer sum)
Data-parallel over batch: core i handles batch row i. No collectives.

Engine mapping (GpSimd deliberately unused -- its pool-library load and
drains cost ~12us on HW):
  host   : tok_h cast to bf16 (halves DMA bytes, tolerance is 2e-2),
           membership matrices atw built in numpy, W/b packed to one row
  PE     : W broadcast across partitions (k=1 matmul), projection rows
           (m=1 matmuls: lhsT=u column, rhs=atw), outer sum (two k=1
           matmuls into the [S,T] PSUM tile)
  DVE    : u = tok . w via fused multiply-reduce, one op per 128-token chunk
  ACT    : PSUM->SBUF copies (dtype casts), bias add on the tgt row
  SP/ACT : DMA queues (tok on sync, atw/wcat on scalar)
"""

import functools

import numpy as np
import ml_dtypes

import concourse.bacc as bacc
import concourse.mybir as mybir
from concourse.bass_utils import run_bass_kernel_spmd
from concourse.tile import TileContext

# Problem geometry (hardcoded per spec)
B = 8
L_SRC = 256
L_TGT = 256
L = L_SRC + L_TGT  # 512
H = 768
P = 128            # SBUF partitions / tokens per chunk
NCHUNK = L // P    # 4
N_SRC_CHUNKS = L_SRC // P  # 2
N_CORES = 8
F32 = mybir.dt.float32
BF16 = mybir.dt.bfloat16
NPBF16 = ml_dtypes.bfloat16


# ---------------------------------------------------------------------------
# Host-side segment bookkeeping (exact mirror of reference._pool_words)
# ---------------------------------------------------------------------------

def _segments(combined_wid, attention_mask, n_words):
    """Per-token dense run ids exactly as the reference computes them."""
    valid = (attention_mask > 0) & (combined_wid >= 0)  # [B, L]
    prev_wid = np.concatenate(
        [np.full((combined_wid.shape[0], 1), -2, dtype=combined_wid.dtype),
         combined_wid[:, :-1]], axis=1)
    prev_valid = np.concatenate(
        [np.zeros((valid.shape[0], 1), dtype=bool), valid[:, :-1]], axis=1)
    new_run = valid & ((combined_wid != prev_wid) | (~prev_valid))
    run_id = np.cumsum(new_run.astype(np.int64), axis=1) - 1  # [B, L]
    seg = np.where(valid, run_id, n_words)  # n_words = dummy slot
    return seg, valid


def _seg_weights(seg, valid, n_words):
    """1/max(count,1) weight for each token's segment (0 for invalid)."""
    Bv, Lv = seg.shape
    wgt = np.zeros((Bv, Lv), dtype=np.float32)
    for b in range(Bv):
        counts = np.bincount(seg[b][valid[b]], minlength=Lv + 1).astype(np.float32)
        inv = 1.0 / np.maximum(counts, 1.0)
        wgt[b] = np.where(valid[b] & (seg[b] < n_words), inv[np.minimum(seg[b], Lv)], 0.0)
    return wgt


# ---------------------------------------------------------------------------
# Device kernel
# ---------------------------------------------------------------------------

def _emit(nc, tc, S, T, block_ok):
    """block_ok fast path: src tokens only map to word rows [0,S), tgt
    tokens only to [S,S+T) -> atw is [L, P] with in-block columns and each
    chunk does ONE reduce.  General path: atw is [L, S+T] and each chunk
    reduces against both weight halves."""
    NW = S + T
    AW = P if block_ok else NW
    tok = nc.declare_dram_parameter("tok", [L, H], BF16, isOutput=False)
    atw = nc.declare_dram_parameter("atw", [L, AW], BF16, isOutput=False)
    # wcat = [w_src (H) | w_tgt (H) | bias (1)] in bf16
    wcat = nc.declare_dram_parameter("wcat", [1, 2 * H + 1], BF16, isOutput=False)
    out = nc.declare_dram_parameter("out", [S, T], F32, isOutput=True)

    with (
        tc.tile_pool(name="const", bufs=1) as cpool,
        tc.tile_pool(name="toks", bufs=1) as tpool,
        tc.tile_pool(name="prods", bufs=2) as ppool,
        tc.tile_pool(name="psum", bufs=1, space="PSUM") as pspool,
    ):
        # wcat rides the ACT DGE queue so the (tiny, pipeline-gating)
        # transfer never queues behind the token chunks.
        wcat_sb = cpool.tile([1, 2 * H + 1], BF16)
        nc.scalar.dma_start(out=wcat_sb[:], in_=wcat[:])

        ones_bf = cpool.tile([1, P], BF16)
        nc.vector.memset(ones_bf[:], 1.0)

        # W broadcast across partitions: k=1 matmul (out[p, j] =
        # ones[0, p] * wcat[0, j]), then ACT copies PSUM -> SBUF bf16.
        wb_sb = cpool.tile([P, 2 * H], BF16)
        for j in range(3):
            wbp = pspool.tile([P, 512], F32, name=f"wbp{j}")
            nc.tensor.matmul(wbp[:], ones_bf[:], wcat_sb[0:1, j * 512:(j + 1) * 512],
                             start=True, stop=True)
            nc.scalar.copy(out=wb_sb[:, j * 512:(j + 1) * 512], in_=wbp[:])

        proj_src_ps = pspool.tile([P, P], F32, name="proj_src")
        proj_tgt_ps = pspool.tile([P, P], F32, name="proj_tgt")

        n_u = NCHUNK if block_ok else 2 * NCHUNK
        u_sb = cpool.tile([P, n_u], F32)
        u_bf = cpool.tile([P, n_u], BF16)

        for c in range(NCHUNK):
            tok_c = tpool.tile([P, H], BF16, name=f"tok{c}")
            nc.sync.dma_start(out=tok_c[:], in_=tok[c * P:(c + 1) * P, :])
            atw_c = tpool.tile([P, AW], BF16, name=f"atw{c}")
            nc.scalar.dma_start(out=atw_c[:], in_=atw[c * P:(c + 1) * P, :])

            if block_ok:
                is_src = c < N_SRC_CHUNKS
                jobs = [(0 if is_src else 1,
                         atw_c[:, 0:(S if is_src else T)],
                         proj_src_ps if is_src else proj_tgt_ps,
                         S if is_src else T,
                         c % N_SRC_CHUNKS == 0, c % N_SRC_CHUNKS == 1, c)]
            else:
                jobs = [(0, atw_c[:, 0:S], proj_src_ps, S,
                         c == 0, c == NCHUNK - 1, 2 * c),
                        (1, atw_c[:, S:NW], proj_tgt_ps, T,
                         c == 0, c == NCHUNK - 1, 2 * c + 1)]

            for half, atw_ap, proj_ps, width, first, last, ui in jobs:
                prod = ppool.tile([P, H], BF16, name=f"prod{ui % 2}")
                nc.vector.affine_mul_reduce(
                    out=prod[:], accum_out=u_sb[:, ui:ui + 1], in0=tok_c[:],
                    in1=wb_sb[:, half * H:(half + 1) * H], scale=1.0, bias=0.0)
                nc.scalar.copy(out=u_bf[:, ui:ui + 1], in_=u_sb[:, ui:ui + 1])
                nc.tensor.matmul(
                    proj_ps[0:1, 0:width], u_bf[:, ui:ui + 1], atw_ap,
                    start=first, stop=last, skip_group_check=True)

        # proj rows -> SBUF bf16; bias is added onto the tgt row here
        rows = cpool.tile([1, 2 * P], BF16)
        nc.scalar.copy(out=rows[0:1, 0:S], in_=proj_src_ps[0:1, 0:S])
        nc.scalar.add(out=rows[0:1, P:P + T], in_=proj_tgt_ps[0:1, 0:T],
                      add=wcat_sb[0:1, 2 * H:2 * H + 1])

        # outer sum: out[s, t] = proj_src[s] + (proj_tgt[t] + b)
        out_ps = pspool.tile([S, T], F32, name="out_ps")
        nc.tensor.matmul(out_ps[:], rows[0:1, 0:S], ones_bf[0:1, 0:T],
                         start=True, stop=False, skip_group_check=True)
        nc.tensor.matmul(out_ps[:], ones_bf[0:1, 0:S], rows[0:1, P:P + T],
                         start=False, stop=True, skip_group_check=True)

        out_sb = cpool.tile([S, T], F32)
        nc.scalar.copy(out=out_sb[:], in_=out_ps[:])
        nc.sync.dma_start(out=out[:], in_=out_sb[:])


@functools.lru_cache(maxsize=4)
def _build(S, T, block_ok):
    nc = bacc.Bacc("TRN2", debug=False, num_devices=N_CORES)
    with TileContext(nc) as tc:
        _emit(nc, tc, S, T, block_ok)
    nc.compile()
    return nc


# ---------------------------------------------------------------------------
# Host wrapper
# ---------------------------------------------------------------------------

def _prep(inputs):
    tok_h = np.ascontiguousarray(np.asarray(inputs["tok_h"], dtype=np.float32))
    mask = np.asarray(inputs["attention_mask"])
    swid = np.asarray(inputs["source_word_ids"])
    twid = np.asarray(inputs["target_word_ids"])
    W = np.asarray(inputs["W"], dtype=np.float32)
    b = np.asarray(inputs["b"], dtype=np.float32)
    S = int(np.asarray(inputs["S"]))
    T = int(np.asarray(inputs["T"]))

    Bv, Lv, Hv = tok_h.shape
    assert (Bv, Lv, Hv) == (B, L, H), f"unexpected tok_h shape {tok_h.shape}"
    assert swid.shape == (B, L_SRC) and twid.shape == (B, L_TGT)
    assert S <= P and T <= P

    NW = S + T
    combined = np.concatenate([swid, twid], axis=1).astype(np.int64)
    seg, valid = _segments(combined, mask, NW)
    wgt = _seg_weights(seg, valid, NW)

    src_tok_seg = seg[:, :L_SRC][valid[:, :L_SRC]]
    tgt_tok_seg = seg[:, L_SRC:][valid[:, L_SRC:]]
    block_ok = bool(
        (src_tok_seg < S).all()
        and (tgt_tok_seg >= S).all() and (tgt_tok_seg < NW).all()
    )

    wcat = np.zeros((1, 2 * H + 1), dtype=np.float32)
    wcat[0, :H] = W[:H, 0]
    wcat[0, H:2 * H] = W[H:2 * H, 0]
    wcat[0, 2 * H] = b.reshape(-1)[0]
    wcat_bf = wcat.astype(NPBF16)

    AW = P if block_ok else NW
    tidx = np.arange(L)
    in_maps = []
    for bi in range(B):
        atw_f = np.zeros((L, AW), dtype=np.float32)
        segb = seg[bi]
        ok = valid[bi] & (segb < NW)
        if block_ok:
            col = np.where(tidx < L_SRC, segb, segb - S)
        else:
            col = segb
        atw_f[tidx[ok], col[ok]] = wgt[bi][ok]
        in_maps.append({
            "tok": tok_h[bi].astype(NPBF16),
            "atw": atw_f.astype(NPBF16),
            "wcat": wcat_bf,
        })
    return S, T, block_ok, in_maps


def kernel(**inputs):
    S, T, block_ok, in_maps = _prep(inputs)
    nc = _build(S, T, block_ok)
    res = run_bass_kernel_spmd(nc, in_maps, core_ids=list(range(N_CORES)))
    return np.stack([res.results[i]["out"] for i in range(B)], axis=0)
